# revision 4
# baseline (speedup 1.0000x reference)
"""ColorHistogramLoss (soft histogram EMD) on 8 Trainium2 NeuronCores.

Strategy: pure data parallel over batch (B=8 -> one batch element per core).
Each core computes, for its 3 channels x {pred, target}, the 64-bin soft
(Gaussian-weighted) histogram of its 384x384 image.

Dense work = 64 Gaussian evals per pixel, split across two engines:

- ACT (scalar): Derivative_Erf(scale*x + bias) = 2/sqrt(pi)*exp(-u^2) with
  fused accum_out free-dim reduction; image laid out [16, FREE] replicated
  8x across partition blocks so a PER-PARTITION bias makes one pass cover
  8 bins (block k evaluates bin 8k+r).  ACT computes per quarter: tau (Exp),
  seeds r=0, r=4, and (for some units) a dense r=7 pass.
- DVE (vector): the Gaussian ratio recurrence
      w_{j+1}(x) = w_j(x) * tau(x) * C_j,   tau = exp((2s/D) x),
      C_j = exp(-s (c_j + c_{j+1})/D)
  via ONE fused scalar_tensor_tensor pass per 8-bin round:
      out = (w_prev * C[per-partition]) * tau,  accum_out = sum(out).
  Chains r=1,2,3 hang off the r=0 seed and r=5,6(,7) off the r=4 seed.
  Chain tensors are BF16: scalar_tensor_tensor then runs in the 2x_1P DVE
  perf mode (2 elem/cycle), halving chain cost; accumulation stays fp32.
  14 of 24 units run 6 DVE rounds (no ACT dense pass), 10 run 5 -- balancing
  ACT (~1.8us/pass + table switches) against DVE (~1.35us/round).

Exp and Derivative_Erf live in different ACT table sets (~2.7us per switch),
so ACT passes are batched per image: [Exp: 4x tau][DErf: seeds/dense].

Work unit = image quarter [128, 2304].  Per-partition partial sums land in
hacc[128, 192]; two PE matmuls against a block selector reduce over the 16
rows of each block; the tiny tail (normalize, cumsum, |diff|, mean) runs on
host in float64.
"""

import functools
import math

import numpy as np

N_CORES = 8
NUM_BINS = 64
B, C, H, W = 8, 3, 384, 384
HW = H * W
N_UNITS = 2 * C                 # (channel, pred/target) images per core
FREE = HW // 16                 # channel image as [16, 9216], replicated 8x
NQ = 4                          # quarters per image
QF = FREE // NQ                 # quarter free dim (2304)
UNITS = N_UNITS * NQ            # 24 work units per core per iteration

DENOM = 2.0 * (1.0 / 64.0) ** 2 + 1e-7
SCALE = 1.0 / math.sqrt(DENOM)          # Derivative_Erf arg scale
SPACING = 1.0 / 63.0                    # bin-center spacing
TAU_SCALE = 2.0 * SPACING / DENOM       # tau = exp(TAU_SCALE * x)
DERF_SCALE = math.sqrt(math.pi) / 2.0   # Derivative_Erf = 2/sqrt(pi)*exp(-u^2)

HACC_COLS = UNITS * 8           # 192

# consts column layout
CB0, CB4, CB7 = 0, 1, 2                     # Derivative_Erf bias r=0,4,7
CC = {1: 3, 2: 4, 3: 5, 5: 6, 6: 7, 7: 8}   # recurrence C for round r
CSEL = 9                                    # selector cols 9..16
NCONST = 20


def _d6(g):
    """Units with 6 DVE rounds (no ACT dense pass)."""
    return (g % 24) % 12 < 7


def _unit_act_rounds(g):
    return ("s0", "s4") if _d6(g) else ("s0", "s4", "d7")


def _unit_dve_rounds(g):
    return ("r1", "r2", "r3", "r5", "r6", "r7") if _d6(g) else (
        "r1", "r2", "r3", "r5", "r6")


def _build_schedules(R):
    """Static per-engine instruction schedules with absolute indices."""
    act_prog, dve_prog = [], []
    for r in range(R):
        for i in range(N_UNITS):
            base = 24 * r + NQ * i
            for q in range(NQ):
                act_prog.append(("tau", base + q))
            for q in range(NQ):
                g = base + q
                act_prog += [(k, g) for k in _unit_act_rounds(g)]
        for u in range(UNITS):
            g = 24 * r + u
            dve_prog += [(k, g) for k in _unit_dve_rounds(g)]
    act_index = {key: idx for idx, key in enumerate(act_prog)}
    dve_index = {key: idx for idx, key in enumerate(dve_prog)}
    return act_prog, dve_prog, act_index, dve_index


def _build_program(R=1):
    import concourse.bass as bass
    import concourse.mybir as mybir
    from contextlib import ExitStack

    act_prog, dve_prog, act_index, dve_index = _build_schedules(R)
    act_per_it = len(act_prog) // R
    dve_per_it = len(dve_prog) // R

    nc = bass.Bass()
    xs = [
        nc.dram_tensor(f"x{u}", [128, FREE], mybir.dt.float32, kind="ExternalInput")
        for u in range(N_UNITS)
    ]
    cst = nc.dram_tensor(
        "consts", [128, NCONST], mybir.dt.float32, kind="ExternalInput"
    )
    hist_out = nc.dram_tensor(
        "hist", [128, 16], mybir.dt.float32, kind="ExternalOutput"
    )

    mult = mybir.AluOpType.mult
    bf16 = mybir.dt.bfloat16

    with ExitStack() as stack:
        def sb(name, shape, dt=mybir.dt.float32):
            return stack.enter_context(nc.sbuf_tensor(name, shape, dt))

        xts = [sb(f"xt{i}", [128, QF]) for i in range(6)]
        taus = [sb(f"tau{i}", [128, QF], bf16) for i in range(4)]
        sas = [sb(f"sa{i}", [128, QF], bf16) for i in range(2)]
        sbs = [sb(f"sb{i}", [128, QF], bf16) for i in range(2)]
        p0s = [sb(f"p0{i}", [128, QF], bf16) for i in range(2)]
        wscr = sb("wscr", [128, QF])
        cstt = sb("cstt", [128, NCONST])
        hacc = sb("hacc", [128, HACC_COLS])
        ho = sb("ho", [128, 16])
        ph0 = stack.enter_context(nc.psum_tensor("ph0", [128, 8], mybir.dt.float32))
        ph1 = stack.enter_context(nc.psum_tensor("ph1", [64, 8], mybir.dt.float32))
        sem_c = stack.enter_context(nc.semaphore("sem_c"))
        xsems = [stack.enter_context(nc.semaphore(f"sem_x{i}")) for i in range(6)]
        act_sem = stack.enter_context(nc.semaphore("act_sem"))
        dve_sem = stack.enter_context(nc.semaphore("dve_sem"))
        pe_sem = stack.enter_context(nc.semaphore("pe_sem"))
        cp_sem = stack.enter_context(nc.semaphore("cp_sem"))
        block = stack.enter_context(nc.Block())

        def col(g, rnd):
            return 8 * (g % 24) + rnd

        def haccol(g, rnd):
            c = col(g, rnd)
            return hacc[:, c : c + 1]

        @block.sync
        def _(sync):
            sync.dma_start(out=cstt[:], in_=cst[:]).then_inc(sem_c, 16)
            for r in range(R):
                for u in range(UNITS):
                    g = 24 * r + u
                    slot = g % 6
                    i, q = (g % 24) // NQ, g % NQ
                    if g >= 6:
                        # xt slot free once unit g-6's last ACT read done
                        last = _unit_act_rounds(g - 6)[-1]
                        sync.wait_ge(act_sem, act_index[(last, g - 6)] + 1)
                    sync.dma_start(
                        out=xts[slot][:], in_=xs[i][:, QF * q : QF * (q + 1)]
                    ).then_inc(xsems[slot], 16)
                sync.wait_ge(cp_sem, 2 * (r + 1))
                sync.dma_start(out=hist_out[:], in_=ho[:]).then_inc(sem_c, 16)

        @block.scalar
        def _(scalar):
            # dummy activation: pulls the exp table load forward
            scalar.activation(
                wscr[0:128, 0:1], wscr[0:128, 1:2],
                mybir.ActivationFunctionType.Exp,
                bias=0.0, scale=1.0,
            )
            scalar.wait_ge(sem_c, 16)
            for kind, g in act_prog:
                slot, q, b = g % 6, g % 4, g % 2
                r = g // 24
                scalar.wait_ge(xsems[slot], 16 * (g // 6 + 1))
                if kind == "tau":
                    if g >= 4:
                        # tau slot free once unit g-4's last DVE read done
                        last = _unit_dve_rounds(g - 4)[-1]
                        scalar.wait_ge(dve_sem, dve_index[(last, g - 4)] + 1)
                    ins = scalar.activation(
                        taus[q][:], xts[slot][:],
                        mybir.ActivationFunctionType.Exp,
                        bias=0.0, scale=float(TAU_SCALE),
                    )
                elif kind == "s0":
                    if g % 24 == 0 and r > 0:
                        # hacc reused across iterations; PE must have read it
                        scalar.wait_ge(pe_sem, r)
                    if g >= 2:
                        # sa bank last touched by DVE r3 of unit g-2
                        scalar.wait_ge(dve_sem, dve_index[("r3", g - 2)] + 1)
                    ins = scalar.activation(
                        sas[b][:], xts[slot][:],
                        mybir.ActivationFunctionType.Derivative_Erf,
                        bias=cstt[:, CB0 : CB0 + 1], scale=float(SCALE),
                        accum_out=haccol(g, 0),
                    )
                elif kind == "s4":
                    if g >= 2:
                        last = "r7" if _d6(g - 2) else "r6"
                        scalar.wait_ge(dve_sem, dve_index[(last, g - 2)] + 1)
                    ins = scalar.activation(
                        sbs[b][:], xts[slot][:],
                        mybir.ActivationFunctionType.Derivative_Erf,
                        bias=cstt[:, CB4 : CB4 + 1], scale=float(SCALE),
                        accum_out=haccol(g, 4),
                    )
                else:  # d7
                    ins = scalar.activation(
                        wscr[:], xts[slot][:],
                        mybir.ActivationFunctionType.Derivative_Erf,
                        bias=cstt[:, CB7 : CB7 + 1], scale=float(SCALE),
                        accum_out=haccol(g, 7),
                    )
                ins.then_inc(act_sem, 1)

        @block.vector
        def _(vector):
            # chain buffers: r1: sa->p0, r2: p0->sa, r3: sa->p0,
            #                r5: sb->p0, r6: p0->sb, r7: sb->p0
            srcdst = {
                "r1": (sas, p0s), "r2": (p0s, sas), "r3": (sas, p0s),
                "r5": (sbs, p0s), "r6": (p0s, sbs), "r7": (sbs, p0s),
            }
            for kind, g in dve_prog:
                q, b = g % 4, g % 2
                rnd = int(kind[1])
                if kind == "r1":
                    vector.wait_ge(act_sem, act_index[("s0", g)] + 1)
                elif kind == "r5":
                    vector.wait_ge(act_sem, act_index[("s4", g)] + 1)
                src, dst = srcdst[kind]
                cc = CC[rnd]
                ins = vector.scalar_tensor_tensor(
                    dst[b][:], src[b][:], cstt[:, cc : cc + 1], taus[q][:],
                    mult, mult,
                    accum_out=haccol(g, rnd),
                )
                ins.then_inc(dve_sem, 1)
                if g % 24 == 23 and kind == _unit_dve_rounds(g)[-1]:
                    # end of iteration: copy PE results out
                    it = g // 24
                    vector.wait_ge(pe_sem, it + 1)
                    vector.wait_ge(sem_c, 16 * (it + 1))
                    vector.tensor_copy(ho[:, 0:8], ph0[:, :]).then_inc(cp_sem, 1)
                    vector.tensor_copy(ho[0:64, 8:16], ph1[:, :]).then_inc(
                        cp_sem, 1
                    )

        @block.tensor
        def _(tensor):
            for r in range(R):
                tensor.wait_ge(act_sem, act_per_it * (r + 1))
                tensor.wait_ge(dve_sem, dve_per_it * (r + 1))
                tensor.matmul(
                    ph0[0:128, 0:8], hacc[:, 0:128], cstt[:, CSEL : CSEL + 8],
                    start=True, stop=True,
                )
                tensor.matmul(
                    ph1[0:64, 0:8], hacc[:, 128:192], cstt[:, CSEL : CSEL + 8],
                    start=True, stop=True,
                ).then_inc(pe_sem, 1)

    return nc


def _make_consts():
    centers = np.linspace(0.0, 1.0, NUM_BINS).astype(np.float64)
    p = np.arange(128)
    k = p // 16
    cst = np.zeros((128, NCONST), dtype=np.float64)
    for ci, r in ((CB0, 0), (CB4, 4), (CB7, 7)):
        cst[:, ci] = -centers[8 * k + r] * SCALE
    for r, ci in CC.items():
        j = 8 * k + r
        cst[:, ci] = np.exp(-SPACING * (centers[j - 1] + centers[j]) / DENOM)
    for kk in range(8):
        cst[k == kk, CSEL + kk] = 1.0
    return cst.astype(np.float32)


@functools.lru_cache(maxsize=1)
def _get_runner():
    """Compile the SPMD program once; return a callable list[in_map] -> list[out_map]."""
    import jax
    from jax.experimental.shard_map import shard_map
    from jax.sharding import Mesh, PartitionSpec

    from concourse import mybir
    from concourse.bass2jax import (
        _bass_exec_p,
        install_neuronx_cc_hook,
        partition_id_tensor,
    )

    nc = _build_program()
    install_neuronx_cc_hook()

    partition_name = (
        nc.partition_id_tensor.name if nc.partition_id_tensor else None
    )
    in_names, out_names, out_avals, zero_outs = [], [], [], []
    for alloc in nc.m.functions[0].allocations:
        if not isinstance(alloc, mybir.MemoryLocationSet):
            continue
        name = alloc.memorylocations[0].name
        if alloc.kind == "ExternalInput":
            if name != partition_name:
                in_names.append(name)
        elif alloc.kind == "ExternalOutput":
            out_names.append(name)
            shape = tuple(alloc.tensor_shape)
            dtype = mybir.dt.np(alloc.dtype)
            out_avals.append(jax.core.ShapedArray(shape, dtype))
            zero_outs.append(np.zeros(shape, dtype))
    n_params = len(in_names)
    n_outs = len(out_avals)
    all_in_names = list(in_names) + list(out_names)
    if partition_name is not None:
        all_in_names.append(partition_name)
    donate = tuple(range(n_params, n_params + n_outs))

    def _body(*args):
        operands = list(args)
        if partition_name is not None:
            operands.append(partition_id_tensor())
        outs = _bass_exec_p.bind(
            *operands,
            out_avals=tuple(out_avals),
            in_names=tuple(all_in_names),
            out_names=tuple(out_names),
            lowering_input_output_aliases=(),
            sim_require_finite=True,
            sim_require_nnan=True,
            nc=nc,
        )
        return tuple(outs)

    devices = jax.devices()[:N_CORES]
    mesh = Mesh(np.asarray(devices), ("core",))
    sharded = jax.jit(
        shard_map(
            _body,
            mesh=mesh,
            in_specs=(PartitionSpec("core"),) * (n_params + n_outs),
            out_specs=(PartitionSpec("core"),) * n_outs,
            check_rep=False,
        ),
        donate_argnums=donate,
        keep_unused=True,
    )

    class Runner:
        def __init__(self):
            self.sharded = sharded
            self.in_names = in_names
            self.out_names = out_names
            self.out_avals = out_avals
            self.zero_outs = zero_outs

        def concat_inputs(self, in_maps):
            return [
                np.concatenate([np.asarray(m[name]) for m in in_maps], axis=0)
                for name in in_names
            ]

        def fresh_zeros(self):
            return [
                np.zeros((N_CORES * z.shape[0], *z.shape[1:]), z.dtype)
                for z in zero_outs
            ]

        def split_outputs(self, out_arrs):
            return [
                {
                    name: np.asarray(out_arrs[i]).reshape(
                        N_CORES, *out_avals[i].shape
                    )[c]
                    for i, name in enumerate(out_names)
                }
                for c in range(N_CORES)
            ]

        def __call__(self, in_maps):
            out_arrs = self.sharded(*self.concat_inputs(in_maps), *self.fresh_zeros())
            return self.split_outputs(out_arrs)

    return Runner()


def _shard_inputs(pred, target):
    cst = _make_consts()
    maps = []
    for b in range(B):
        m = {"consts": cst}
        for c in range(C):
            for t, src in enumerate((pred, target)):
                u = 2 * c + t
                img = np.ascontiguousarray(src[b, c], dtype=np.float32).reshape(
                    16, FREE
                )
                m[f"x{u}"] = np.tile(img, (8, 1))
        maps.append(m)
    return maps


def _unpack_hist(ho):
    """ho [128, 16] -> hist [N_UNITS, NUM_BINS] (float64)."""
    ho = ho.astype(np.float64)
    hist = np.zeros((N_UNITS, NUM_BINS), dtype=np.float64)
    for u in range(UNITS):
        i = u // NQ
        for rnd in range(8):
            c = 8 * u + rnd
            vals = ho[c, 0:8] if c < 128 else ho[c - 128, 8:16]
            for k in range(8):
                hist[i, 8 * k + rnd] += vals[k]
    return hist


def _finish_on_host(results):
    total = 0.0
    for b in range(B):
        hist = _unpack_hist(results[b]["hist"]) * DERF_SCALE
        for c in range(C):
            pcs = hist[2 * c]
            tcs = hist[2 * c + 1]
            pn = pcs / (pcs.sum() + 1e-7)
            tn = tcs / (tcs.sum() + 1e-7)
            total += np.abs(np.cumsum(pn) - np.cumsum(tn)).sum()
    return np.float32(total / (B * C * NUM_BINS))


def kernel(pred, target):
    pred = np.asarray(pred, dtype=np.float32)
    target = np.asarray(target, dtype=np.float32)
    assert pred.shape == (B, C, H, W) and target.shape == (B, C, H, W)
    run = _get_runner()
    results = run(_shard_inputs(pred, target))
    return np.asarray(_finish_on_host(results), dtype=np.float32)
